# revision 25
# baseline (speedup 1.0000x reference)
"""DISCO S2 conv kernel for 8 trn2 NeuronCores (self-contained).

Math: out[bc,k,t,p] = sum_j val_j*qw[h_j]*x[bc,h_j,(lon_j-2p-2) mod 720]
Parity/reversal transform: lon=2e+par ->
  out[bc,k,t,p] = sum v * xr_par[h, (p+1-e) mod 360, bc],
  xr_par[h,m] = x[bc,h, 2*((-m) mod 360)+par].
Per latitude chunk m in [60c,60c+60): p = (60c + jj - B - 1) mod 360 with
jj = m_rel + e_s + B  (e_s signed lon/2, |e_s|<=B per class).
Matmul: psum[bc, (jj,k)] += sum_{par,m_rel} xtile[(par,m_rel), bc] * M[(par,m_rel),(jj,k)]
  xtile = stationary (lhsT), M = Toeplitz master (moving rhs).

v2: N region runs entirely in fp8 e4m3 DoubleRow perf mode (0.5 cyc/col)
with value+residual compensation: x ~ x4 + xr, m ~ m4 + mr (all e4m3),
out = x4*m4 + xr*m4 + x4*mr (xr*mr dropped; ~0.4% of |xm|). DR packs the
6 r-rows as 3 r-pair passes -> 9 DR passes per chunk vs 6 fp16 passes,
at half the per-col cost. N psum blocks ship raw fp32 via PSUM->DRAM DMA
(one strided DMA per slot); the host does the halo overlap-add + unscale,
removing the on-chip retire (ACT/DVE) work for N entirely.
W/M1/M2 polar regions keep the v1 scheme: e4m3 x e4m3 DR, fp16 staged out.

Sharding: latitude classes, identical static schedule on all 8 cores
(per-core data differs): per core 1 wide slot (B=180, polar), 1 mid1
(B=48), 2 mid2 (B=15), 19 narrow (B<=7, fp8-pair) slots.
"""

import numpy as np

# ---------------- problem constants (hardcoded) ----------------
B_, C_ = 2, 64
BC = 128
H, W = 360, 720
T, P, KS = 180, 360, 3
M360 = 360

# classes: (name, B, t-list global, slots per core)
W_TS = [0, 1, 178, 179]          # 4 real + 4 dummy -> 1 slot/core
M1_TS = [2, 3, 4, 5, 174, 175, 176, 177]   # 1 slot/core
M2A_TS = list(range(6, 14))      # 1 slot/core
M2B_TS = list(range(166, 174))   # 1 slot/core
N_T0 = 14                        # N-class t = 14 + 19*core + i, i in [0,19)
N_PER_CORE = 19
N_PAIRS = N_PER_CORE + 2         # r-pair slab tiles per core (pairs i..i+2 per slot)

B_W, B_M1, B_M2, B_N = 180, 48, 15, 7
WIN_M2 = 60 + 2 * B_M2   # 90
WIN_M1 = 60 + 2 * B_M1   # 156 ; par-fused Kc=60 (per-chunk psum blocks)
WIN_W = 360              # tail-folded full circle (W split into k-pieces)
N_CORES = 8

F32 = np.float32
# Precision plan: polar (W/M1/M2) e4m3 x e4m3 DoubleRow as in v1. N region
# e4m3 pairs (value+residual) for both x and masters, all DR. x pre-scaled
# by XS, masters by MSCALE; output divided back.
IO_DTYPE = "float16"
MSCALE = 131072.0
XS = 2.0


def _cc_lats(n):
    return np.pi * np.arange(n) / (n - 1)


def _entries(psi_seg, psi_lat, psi_lon, psi_val, quad_weights):
    seg = np.asarray(psi_seg); lat = np.asarray(psi_lat)
    lon = np.asarray(psi_lon); val = np.asarray(psi_val, dtype=np.float64)
    qw = np.asarray(quad_weights, dtype=np.float64).reshape(-1)
    t = seg % T
    k = seg // T
    h = lat
    r = h - 2 * t + 2
    par = lon % 2
    e = lon // 2
    e_s = np.where(e < 180, e, e - 360)
    v = (val * qw[h]).astype(F32)
    return t, k, h, r, par, e_s, v


def _derive_tables(ent):
    """Per-slot-position band widths (identical across cores, data-derived)."""
    t, k, h, r, par, e_s, v = ent
    ext = np.zeros(T, dtype=int)
    np.maximum.at(ext, t, np.abs(e_s))
    n_bi = [max(int(ext[N_T0 + N_PER_CORE * c + i]) for c in range(N_CORES))
            for i in range(N_PER_CORE)]
    b_m2 = [max(int(ext[tv]) for tv in M2A_TS), max(int(ext[tv]) for tv in M2B_TS)]
    assert max(n_bi) <= B_N and max(b_m2) <= B_M2
    assert max(int(ext[tv]) for tv in M1_TS) <= B_M1
    return {"n_bi": n_bi, "b_m2": b_m2}


def _w_pieces(ci):
    """Two W k-pieces per core: (t, k) or None (dummy)."""
    if ci < 4:
        return [(W_TS[ci], 0), (W_TS[ci], 1)]
    return [(W_TS[ci - 4], 2), None]


def _core_assignment(ci):
    """slot -> t (or None for dummy). slots: 0=W,1=M1,2=M2a,3=M2b,4..22=N."""
    ts = [None] * 23
    if ci < len(W_TS):
        ts[0] = W_TS[ci]
    ts[1] = M1_TS[ci]
    ts[2] = M2A_TS[ci]
    ts[3] = M2B_TS[ci]
    for i in range(N_PER_CORE):
        ts[4 + i] = N_T0 + N_PER_CORE * ci + i
    return ts


def _precompute_globals(xbc, ent):
    """One-time global tensors: reversed parity-split x (fp32, cast at pack
    time) + class master banks (N fp32 fused; M1/W/M2 e4m3 DR-paired)."""
    import ml_dtypes
    E4 = ml_dtypes.float8_e4m3
    ms = MSCALE
    t, k, h, r, par, e_s, v = ent
    midx = (-np.arange(M360)) % M360
    # XR[par][h, m_rel(60), chunk(6)... stored [h, 6s, 60m, 128bc]
    XRE = np.ascontiguousarray(
        xbc[:, :, 2 * midx].transpose(1, 2, 0).reshape(H, 6, 60, BC))
    XRO = np.ascontiguousarray(
        xbc[:, :, 2 * midx + 1].transpose(1, 2, 0).reshape(H, 6, 60, BC))

    mr60 = np.arange(60)

    def fused_bank(t_list, Bc, width):
        tidx = {tv: i for i, tv in enumerate(t_list)}
        bank = np.zeros((len(t_list), 6, 120, 3 * width), dtype=F32)
        sel = np.isin(t, t_list)
        ti = np.array([tidx[tv] for tv in t[sel]])
        rr, pp, kk, ee, vv = r[sel], par[sel], k[sel], e_s[sel], v[sel]
        rows = (pp[:, None] * 60 + mr60[None, :]).ravel()
        jj = (mr60[None, :] + ee[:, None] + Bc).ravel()
        cols = jj * 3 + np.repeat(kk, 60)
        np.add.at(bank, (np.repeat(ti, 60), np.repeat(rr, 60), rows, cols),
                  np.repeat(vv, 60))
        # -> [n, 120p, 6r, 3W] contiguous, scaled
        return np.ascontiguousarray(bank.transpose(0, 2, 1, 3) * ms)

    # N masters stay fp32 here; per-core packing cuts per-slot width and
    # quantizes to e4m3 value+residual pairs.
    bank_N = fused_bank(list(range(N_T0, N_T0 + N_PER_CORE * N_CORES)),
                        B_N, 60 + 2 * B_N)
    bank_M2 = fused_bank(M2A_TS + M2B_TS, B_M2, WIN_M2).reshape(
        len(M2A_TS + M2B_TS), 120, 3, 2, 3 * WIN_M2).astype(E4)
    # M1: [n, 120, 6r, 468] -> DR pairs [n, 120, 3pair, 2plane, 468] e4m3
    bank_M1 = fused_bank(M1_TS, B_M1, WIN_M1)
    bank_M1 = bank_M1.reshape(len(M1_TS), 120, 3, 2, 3 * WIN_M1).astype(E4)

    # W pieces: single-k folded [t(4), k(3), 120p, 6r, 360]
    bank_W = np.zeros((4, 3, 6, 120, 360), dtype=F32)
    tidx = {tv: i for i, tv in enumerate(W_TS)}
    sel = np.isin(t, W_TS)
    ti = np.array([tidx[tv] for tv in t[sel]])
    rr, pp, kk, ee, vv = r[sel], par[sel], k[sel], e_s[sel], v[sel]
    rows = (pp[:, None] * 60 + mr60[None, :]).ravel()
    jj = ((mr60[None, :] + ee[:, None] + B_W) % 360).ravel()
    np.add.at(bank_W, (np.repeat(ti, 60), np.repeat(kk, 60), np.repeat(rr, 60),
                       rows, jj), np.repeat(vv, 60))
    # [t, k, 120p, 6r, 360] -> DR pairs [t, k, 120, 3, 2, 360] e4m3
    bank_W = np.ascontiguousarray(bank_W.transpose(0, 1, 3, 2, 4) * ms)
    bank_W = bank_W.reshape(4, 3, 120, 3, 2, 360).astype(E4)
    return XRE, XRO, bank_N, bank_M2, bank_M1, bank_W


def _pack_core_inputs(ci, glob, tables):
    import ml_dtypes
    E4 = ml_dtypes.float8_e4m3
    XRE, XRO, bank_N, bank_M2, bank_M1, bank_W = glob
    ts = _core_assignment(ci)

    # ---- polar slab: rows h = 2t+r-2 per region (W,M1,M2a,M2b), e4m3 pairs
    w_t = W_TS[ci % 4]
    polar_rows = []
    for slot, tv in ((0, w_t), (1, ts[1]), (2, ts[2]), (3, ts[3])):
        for rr in range(6):
            hh = 2 * tv + rr - 2
            polar_rows.append(hh if 0 <= hh < H else None)
    slabp = np.zeros((24, 128, 6, 128), dtype=F32)
    for j, hh in enumerate(polar_rows):
        if hh is None:
            continue
        slabp[j, 0:60] = XRE[hh].transpose(1, 0, 2)
        slabp[j, 60:120] = XRO[hh].transpose(1, 0, 2)
    slabp *= XS
    # -> e4m3 DR pair tiles, partition-major [128, 12 pair, 2, 6, 128]
    slab4 = np.ascontiguousarray(
        slabp.reshape(12, 2, 128, 6, 128).transpose(2, 0, 1, 3, 4)).astype(E4)

    # ---- N slab: 21 r-pair tiles [128parm, 2plane, 6chunk, 128bc], x4+xr
    tN0 = ts[4]
    slabn = np.zeros((N_PAIRS, 128, 2, 6, 128), dtype=F32)
    for pj in range(N_PAIRS):
        for pl in range(2):
            hh = 2 * (tN0 - 1 + pj) + pl
            # [6s,60m,128bc] -> parm=(par,60m): rows 0:60 E, 60:120 O
            slabn[pj, 0:60, pl] = XRE[hh].transpose(1, 0, 2)
            slabn[pj, 60:120, pl] = XRO[hh].transpose(1, 0, 2)
    slabn *= XS
    slab_x4 = slabn.astype(E4)
    slab_xr = (slabn - slab_x4.astype(F32)).astype(E4)
    # host layout [128, 21, 2, 6, 128] (partition-major)
    slab_x4 = np.ascontiguousarray(slab_x4.transpose(1, 0, 2, 3, 4))
    slab_xr = np.ascontiguousarray(slab_xr.transpose(1, 0, 2, 3, 4))

    # ---- N masters: per slot [120, 3rpair, 2plane, 3Wi] e4m3 pairs, concat
    n_bi = tables["n_bi"]
    m4_parts, mr_parts = [], []
    for i in range(N_PER_CORE):
        d = B_N - n_bi[i]
        Wi = 60 + 2 * n_bi[i]
        blk = bank_N[tN0 - N_T0 + i][:, :, 3 * d:3 * (d + Wi)]  # [120,6,3Wi]
        blk = blk.reshape(120, 3, 2, 3 * Wi)
        b4 = blk.astype(E4)
        br = (blk - b4.astype(F32)).astype(E4)
        m4_parts.append(np.ascontiguousarray(b4).reshape(120, -1))
        mr_parts.append(np.ascontiguousarray(br).reshape(120, -1))
    mN4 = np.ascontiguousarray(np.concatenate(m4_parts, axis=1))
    mNr = np.ascontiguousarray(np.concatenate(mr_parts, axis=1))

    # ---- polar masters (unchanged from v1)
    wp = _w_pieces(ci)
    mW = np.zeros((2, 120, 3, 2, 360), dtype=E4)
    widx = {tv: i for i, tv in enumerate(W_TS)}
    for j, piece in enumerate(wp):
        if piece is not None:
            tW, kW = piece
            mW[j] = bank_W[widx[tW], kW]
    mM1 = bank_M1[M1_TS.index(ts[1])]
    b_m2 = tables["b_m2"]
    m2idx = {tv: i for i, tv in enumerate(M2A_TS + M2B_TS)}
    m2_parts = []
    for sl in range(2):
        d = B_M2 - b_m2[sl]
        blk = bank_M2[m2idx[ts[2 + sl]]][:, :, :, 3 * d:3 * (d + 60 + 2 * b_m2[sl])]
        m2_parts.append(np.ascontiguousarray(blk).reshape(120, -1))
    mM2 = np.ascontiguousarray(np.concatenate(m2_parts, axis=1))
    return {"slab4": slab4, "slabx4": slab_x4, "slabxr": slab_xr,
            "mW": mW, "mM1": mM1, "mM2": mM2, "mN4": mN4, "mNr": mNr}


# ---------------- bass kernel emission ----------------

def _emit_kernel(tables):
    import concourse.tile as tile
    from concourse import bacc, mybir
    from contextlib import ExitStack

    DT = mybir.dt.float16
    E4 = mybir.dt.float8e4
    DR = mybir.MatmulPerfMode.DoubleRow
    F32d = mybir.dt.float32
    ADD = mybir.AluOpType.add

    n_bi = tables["n_bi"]
    b_m2 = tables["b_m2"]
    n_w = [60 + 2 * b for b in n_bi]
    m2_w = [60 + 2 * b for b in b_m2]
    n_off = np.concatenate([[0], np.cumsum([3 * 2 * 3 * w for w in n_w])]).tolist()
    m2_off = np.concatenate([[0], np.cumsum([6 * 3 * w for w in m2_w])]).tolist()

    nc = bacc.Bacc(None, target_bir_lowering=False)
    slab4_t = nc.dram_tensor("slab4", [128, 12, 2, 6, 128], E4,
                             kind="ExternalInput")
    slabx4_t = nc.dram_tensor("slabx4", [128, N_PAIRS, 2, 6, 128], E4,
                              kind="ExternalInput")
    slabxr_t = nc.dram_tensor("slabxr", [128, N_PAIRS, 2, 6, 128], E4,
                              kind="ExternalInput")
    mW_t = nc.dram_tensor("mW", [2, 120, 3, 2, 360], E4, kind="ExternalInput")
    mM1_t = nc.dram_tensor("mM1", [120, 3, 2, 3 * WIN_M1], E4,
                           kind="ExternalInput")
    mM2_t = nc.dram_tensor("mM2", [120, m2_off[-1]], E4, kind="ExternalInput")
    mN4_t = nc.dram_tensor("mN4", [120, n_off[-1]], E4, kind="ExternalInput")
    mNr_t = nc.dram_tensor("mNr", [120, n_off[-1]], E4, kind="ExternalInput")
    OUT_DT = DT
    out_t = nc.dram_tensor("out", [4, 128, 3, 360], OUT_DT, kind="ExternalOutput")
    outW_t = nc.dram_tensor("outW", [2, 128, 360], OUT_DT, kind="ExternalOutput")
    outN_t = nc.dram_tensor("outN", [N_PER_CORE, 128, 3, 512], OUT_DT,
                            kind="ExternalOutput")

    def retire_add(stage, ps, jj_len, p0, cov=None):
        """stage[:,:,p] (+)= ps cols; fresh p-cols (per cov map) use an ACT
        copy, already-covered cols a DVE add. cov=None -> all adds."""
        segs = []
        if cov is None:
            segs = [(0, jj_len, True)]
        else:
            a = 0
            while a < jj_len:
                p = (p0 + a) % 360
                old = bool(cov[p])
                b = a
                while b < jj_len and bool(cov[(p0 + b) % 360]) == old:
                    b += 1
                segs.append((a, b, old))
                a = b
            for a, b, old in segs:
                for q in range(a, b):
                    cov[(p0 + q) % 360] = True
        for a, b, old in segs:
            done = a
            while done < b:
                pstart = (p0 + done) % 360
                ln = min(b - done, 360 - pstart)
                s = ps[:, done * 3:(done + ln) * 3].rearrange("p (j k) -> p k j", k=3)
                dst = stage[:, :, pstart:pstart + ln]
                if old:
                    nc.vector.tensor_tensor(dst, s, dst, ADD)
                else:
                    nc.vector.tensor_copy(dst, s)
                done += ln

    with tile.TileContext(nc, pool_alloc_mode="queue") as tc:
        with ExitStack() as ctx:
            rows_pool = ctx.enter_context(tc.tile_pool(name="rows", bufs=1))
            stpool = ctx.enter_context(tc.tile_pool(name="stpool", bufs=7))
            nslab_pool = ctx.enter_context(tc.tile_pool(name="nslab", bufs=1))
            nmast_pool = ctx.enter_context(tc.tile_pool(name="nmast", bufs=1))

            # polar slab: one partition-major tile, chunked loads in
            # first-use order (M2 sl0 pair, rest of M2, W, M1)
            p4t = rows_pool.tile([128, 12, 2, 6, 128], E4, name="p4t", tag="xp")
            for a, b in ((6, 7), (7, 9), (9, 12), (0, 3), (3, 6)):
                nc.sync.dma_start(out=p4t[:, a:b], in_=slab4_t[:, a:b])
            pair_tiles = {pj: None for pj in range(12)}

            class _PairView:
                def __getitem__(self, pj):
                    return p4t[:, pj]
            pair_tiles = _PairView()

            # ---- N big tiles; loads split/staggered across the 3 queues.
            x4t = nslab_pool.tile([128, N_PAIRS, 2, 6, 128], E4, name="x4t",
                                  tag="nx4")
            xrt = nslab_pool.tile([128, N_PAIRS, 2, 6, 128], E4, name="xrt",
                                  tag="nxr")
            m4t = nmast_pool.tile([120, n_off[-1]], E4, name="m4t", tag="nm4")
            mrt = nmast_pool.tile([120, n_off[-1]], E4, name="mrt", tag="nmr")
            # Pool queue: N masters (then N out DMAs later)
            nc.gpsimd.dma_start(out=m4t[:, 0:n_off[6]], in_=mN4_t[:, 0:n_off[6]])
            nc.gpsimd.dma_start(out=mrt[:, 0:n_off[6]], in_=mNr_t[:, 0:n_off[6]])
            nc.gpsimd.dma_start(out=m4t[:, n_off[6]:], in_=mN4_t[:, n_off[6]:])
            nc.gpsimd.dma_start(out=mrt[:, n_off[6]:], in_=mNr_t[:, n_off[6]:])

            # ---- polar masters: all ACT DMA gens issued before ACT compute,
            # ordered by first-use time (M2 sl0 split in thirds for fast start)
            pmast_pool = ctx.enter_context(tc.tile_pool(name="pmast", bufs=1))
            m2ts = []
            for sl in range(2):
                Wsl = m2_w[sl]
                m2t = pmast_pool.tile([120, 6 * 3 * Wsl], E4, name=f"m2t{sl}",
                                      tag=f"m2_{sl}")
                if sl == 0:
                    for pr in range(3):
                        nc.scalar.dma_start(
                            out=m2t[:, pr * 6 * Wsl:(pr + 1) * 6 * Wsl],
                            in_=mM2_t[:, m2_off[sl] + pr * 6 * Wsl:
                                      m2_off[sl] + (pr + 1) * 6 * Wsl])
                else:
                    nc.scalar.dma_start(out=m2t[:],
                                        in_=mM2_t[:, m2_off[sl]:m2_off[sl + 1]])
                m2ts.append(m2t)
            mwts = []
            for j in range(2):
                mwt = pmast_pool.tile([120, 3, 2, 360], E4, name=f"mwt{j}",
                                      tag=f"mw_{j}")
                nc.scalar.dma_start(out=mwt[:], in_=mW_t[j])
                mwts.append(mwt)
            m1t = pmast_pool.tile([120, 3, 2, 3 * WIN_M1], E4, name="m1t",
                                  tag="m1")
            nc.scalar.dma_start(out=m1t[:], in_=mM1_t[:, :, :, :])
            nc.scalar.dma_start(out=x4t[:, 0:3], in_=slabx4_t[:, 0:3])
            nc.scalar.dma_start(out=xrt[:, 0:3], in_=slabxr_t[:, 0:3])

            pspool = ctx.enter_context(
                tc.tile_pool(name="pspool", bufs=5, space="PSUM"))
            npspool = ctx.enter_context(
                tc.tile_pool(name="npspool", bufs=3, space="PSUM"))
            # ---------------- M2 regions (slots 2,3): par-fused Kc=60 --------
            if True:
                for sl in range(2):
                    Wsl = m2_w[sl]
                    m2t = m2ts[sl]
                    stage2 = stpool.tile([128, 3, 360], OUT_DT, name=f"stage2_{sl}", tag="st")
                    cov2 = np.zeros(360, dtype=bool)
                    ps2 = [pspool.tile([128, 512], F32d, name=f"m2ps{sl}_{b}", tag="ps")
                           for b in range(3)]
                    for pr in range(3):
                        xt = pair_tiles[6 + sl * 3 + pr]
                        for c in range(6):
                            b, off = c // 2, (c % 2) * 60
                            rhs2 = m2t[:, pr * 6 * Wsl:(pr + 1) * 6 * Wsl]
                            rhs2 = rhs2.rearrange("p (two f) -> p two f", two=2)
                            nc.tensor.matmul(
                                ps2[b][:, off * 3:off * 3 + 3 * Wsl],
                                lhsT=xt[0:120, :, c, :],
                                rhs=rhs2,
                                start=(pr == 0 and c % 2 == 0),
                                stop=(pr == 2 and c % 2 == 1),
                                perf_mode=DR,
                            )
                    for b in range(3):
                        p0 = (120 * b - b_m2[sl] - 1) % 360
                        retire_add(stage2, ps2[b], 60 + Wsl, p0, cov2)
                    nc.scalar.dma_start(out=out_t[2 + sl], in_=stage2[:])

                # ---------------- W region: 2 single-k pieces, DR pairs ------
                if True:
                    wpairs = [pair_tiles[pj] for pj in range(3)]
                    for j in range(2):
                        mwt = mwts[j]
                        stageW = stpool.tile([128, 360], OUT_DT, name=f"stageW{j}", tag="stw")
                        for c in range(6):
                            wps = pspool.tile([128, 512], F32d, name=f"wps{j}_{c}", tag="ps")
                            for pr in range(3):
                                nc.tensor.matmul(
                                    wps[:, 0:360],
                                    lhsT=wpairs[pr][0:120, :, c, :],
                                    rhs=mwt[:, pr, :, :],
                                    start=(pr == 0),
                                    stop=(pr == 2),
                                    perf_mode=DR,
                                )
                            # p = (60c + jj' - 181) mod 360 over jj' in [0,360)
                            p0 = (60 * c - B_W - 1) % 360
                            done = 0
                            while done < 360:
                                pstart = (p0 + done) % 360
                                ln = min(360 - done, 360 - pstart)
                                dst = stageW[:, pstart:pstart + ln]
                                if c == 0:
                                    nc.vector.tensor_copy(dst, wps[:, done:done + ln])
                                else:
                                    nc.vector.tensor_tensor(dst, wps[:, done:done + ln],
                                                            dst, ADD)
                                done += ln
                        nc.scalar.dma_start(out=outW_t[j], in_=stageW[:])
                # ---------------- M1 region (slot 1): DR pairs, 6 psum blocks
                if True:
                    stage1 = stpool.tile([128, 3, 360], OUT_DT, name="stage1", tag="st")
                    cov1 = np.zeros(360, dtype=bool)
                    for rnd in range(2):
                        cs = (0, 1, 2) if rnd == 0 else (3, 4, 5)
                        ps1 = [pspool.tile([128, 512], F32d, name=f"m1ps{c}", tag="ps")
                               for c in cs]
                        for pr in range(3):
                            xt = pair_tiles[3 + pr]
                            for ci, c in enumerate(cs):
                                nc.tensor.matmul(
                                    ps1[ci][:, 0:3 * WIN_M1],
                                    lhsT=xt[0:120, :, c, :],
                                    rhs=m1t[:, pr, :, :],
                                    start=(pr == 0),
                                    stop=(pr == 2),
                                    perf_mode=DR,
                                )
                        for ci, c in enumerate(cs):
                            p0 = (60 * c - B_M1 - 1) % 360
                            retire_add(stage1, ps1[ci], WIN_M1, p0, cov1)
                    nc.scalar.dma_start(out=out_t[1], in_=stage1[:])

            # ---- N slab loads (queued behind polar pair loads on SP queue)
            nc.sync.dma_start(out=x4t[:, 3:8], in_=slabx4_t[:, 3:8])
            nc.sync.dma_start(out=xrt[:, 3:8], in_=slabxr_t[:, 3:8])
            nc.sync.dma_start(out=x4t[:, 8:14], in_=slabx4_t[:, 8:14])
            nc.sync.dma_start(out=xrt[:, 8:14], in_=slabxr_t[:, 8:14])
            nc.sync.dma_start(out=x4t[:, 14:N_PAIRS], in_=slabx4_t[:, 14:N_PAIRS])
            nc.sync.dma_start(out=xrt[:, 14:N_PAIRS], in_=slabxr_t[:, 14:N_PAIRS])

            # ---------------- N region: e4m3 pairs, DoubleRow, psum->DRAM ---
            if True:
                out_engs = [nc.gpsimd, nc.gpsimd]
                for i in range(N_PER_CORE):
                    Wi = n_w[i]
                    used = 180 + 3 * Wi
                    stN = stpool.tile([128, 3, used], OUT_DT, name=f"stN{i}",
                                      tag="stn")
                    for b in range(3):
                        ps = npspool.tile([128, 512], F32d, name=f"nps{i}_{b}",
                                          tag="nps")
                        for c in (2 * b, 2 * b + 1):
                            off = (c % 2) * 180
                            for gi, (xs, ms) in enumerate(
                                    ((x4t, m4t), (x4t, mrt), (xrt, m4t))):
                                for j in range(3):
                                    rhs = ms[:, n_off[i] + j * 2 * 3 * Wi:
                                             n_off[i] + (j + 1) * 2 * 3 * Wi]
                                    rhs = rhs.rearrange("p (two f) -> p two f", two=2)
                                    nc.tensor.matmul(
                                        ps[:, off:off + 3 * Wi],
                                        lhsT=xs[0:120, i + j, :, c, :],
                                        rhs=rhs,
                                        start=(c == 2 * b and gi == 0 and j == 0),
                                        stop=(c == 2 * b + 1 and gi == 2 and j == 2),
                                        perf_mode=DR,
                                    )
                        # retire bank b while bank b+1 matmuls run
                        if (i + b) % 2 == 0:
                            nc.scalar.copy(out=stN[:, b, :], in_=ps[:, 0:used])
                        else:
                            nc.vector.tensor_copy(stN[:, b, :], ps[:, 0:used])
                    if i >= N_PER_CORE - 2:
                        # tail: ship banks separately so the DMA of banks 0-1
                        # overlaps bank 2's matmuls/copy
                        for b in range(3):
                            nc.sync.dma_start(out=outN_t[i][:, b, 0:used],
                                              in_=stN[:, b, :])
                    else:
                        nc.gpsimd.dma_start(out=outN_t[i][:, :, 0:used],
                                            in_=stN[:])
    nc.finalize()
    return nc


_NC_CACHE = {}


def kernel(**inputs) -> np.ndarray:
    x = np.asarray(inputs["x"], dtype=F32)
    ent = _entries(inputs["psi_seg"], inputs["psi_lat"], inputs["psi_lon"],
                   inputs["psi_val"], inputs["quad_weights"])
    xbc = np.ascontiguousarray(x.reshape(BC, H, W))

    tables = _derive_tables(ent)
    glob = _precompute_globals(xbc, ent)
    in_maps = [_pack_core_inputs(ci, glob, tables) for ci in range(N_CORES)]

    key = (tuple(tables["n_bi"]), tuple(tables["b_m2"]))
    if _NC_CACHE.get("key") != key:
        _NC_CACHE["nc"] = _emit_kernel(tables)
        _NC_CACHE["key"] = key
    nc = _NC_CACHE["nc"]

    from concourse.bass_utils import run_bass_kernel_spmd
    try:
        res = run_bass_kernel_spmd(nc, in_maps, core_ids=list(range(N_CORES)))
    except (ImportError, ModuleNotFoundError):
        # BASS_TRACE set but the axon NTFF hook is unavailable in this env
        import os
        os.environ["BASS_NEVER_TRACE"] = "1"
        res = run_bass_kernel_spmd(nc, in_maps, core_ids=list(range(N_CORES)))
    global LAST_RESULTS
    LAST_RESULTS = res

    inv = np.float32(1.0 / (MSCALE * XS))
    n_bi = tables["n_bi"]
    full = np.zeros((BC, KS, T, P), dtype=F32)
    for ci in range(N_CORES):
        o = np.asarray(res.results[ci]["out"])
        ow = np.asarray(res.results[ci]["outW"])
        on = np.asarray(res.results[ci]["outN"])
        ts = _core_assignment(ci)
        for slot in (1, 2, 3):
            tv = ts[slot]
            if tv is not None:
                full[:, :, tv, :] = o[slot].astype(F32) * inv
        for j, piece in enumerate(_w_pieces(ci)):
            if piece is not None:
                tW, kW = piece
                full[:, kW, tW, :] = ow[j].astype(F32) * inv
        # N slots: host overlap-add of raw psum blocks
        for i in range(N_PER_CORE):
            tv = ts[4 + i]
            bi = n_bi[i]
            run = 120 + 2 * bi              # p-run length per block
            acc = np.zeros((BC, KS, P), dtype=F32)
            for b in range(3):
                blk = on[i, :, b, 0:3 * run].reshape(BC, run, KS)
                p0 = (120 * b - bi - 1) % 360
                pidx = (p0 + np.arange(run)) % 360
                np.add.at(acc, (slice(None), slice(None), pidx),
                          blk.transpose(0, 2, 1))
            full[:, :, tv, :] = acc * inv
    return full.reshape(B_, C_, KS, T, P)


# revision 26
# speedup vs baseline: 1.0134x; 1.0134x over previous
"""DISCO S2 conv kernel for 8 trn2 NeuronCores (self-contained).

Math: out[bc,k,t,p] = sum_j val_j*qw[h_j]*x[bc,h_j,(lon_j-2p-2) mod 720]
Parity/reversal transform: lon=2e+par ->
  out[bc,k,t,p] = sum v * xr_par[h, (p+1-e) mod 360, bc],
  xr_par[h,m] = x[bc,h, 2*((-m) mod 360)+par].
Per latitude chunk m in [60c,60c+60): p = (60c + jj - B - 1) mod 360 with
jj = m_rel + e_s + B  (e_s signed lon/2, |e_s|<=B per class).
Matmul: psum[bc, (jj,k)] += sum_{par,m_rel} xtile[(par,m_rel), bc] * M[(par,m_rel),(jj,k)]
  xtile = stationary (lhsT), M = Toeplitz master (moving rhs).

v2: N region runs entirely in fp8 e4m3 DoubleRow perf mode (0.5 cyc/col)
with value+residual compensation: x ~ x4 + xr, m ~ m4 + mr (all e4m3),
out = x4*m4 + xr*m4 + x4*mr (xr*mr dropped; ~0.4% of |xm|). DR packs the
6 r-rows as 3 r-pair passes -> 9 DR passes per chunk vs 6 fp16 passes,
at half the per-col cost. N psum blocks ship raw fp32 via PSUM->DRAM DMA
(one strided DMA per slot); the host does the halo overlap-add + unscale,
removing the on-chip retire (ACT/DVE) work for N entirely.
W/M1/M2 polar regions keep the v1 scheme: e4m3 x e4m3 DR, fp16 staged out.

Sharding: latitude classes, identical static schedule on all 8 cores
(per-core data differs): per core 1 wide slot (B=180, polar), 1 mid1
(B=48), 2 mid2 (B=15), 19 narrow (B<=7, fp8-pair) slots.
"""

import numpy as np

# ---------------- problem constants (hardcoded) ----------------
B_, C_ = 2, 64
BC = 128
H, W = 360, 720
T, P, KS = 180, 360, 3
M360 = 360

# classes: (name, B, t-list global, slots per core)
W_TS = [0, 1, 178, 179]          # 4 real + 4 dummy -> 1 slot/core
M1_TS = [2, 3, 4, 5, 174, 175, 176, 177]   # 1 slot/core
M2A_TS = list(range(6, 14))      # 1 slot/core
M2B_TS = list(range(166, 174))   # 1 slot/core
N_T0 = 14                        # N-class t = 14 + 19*core + i, i in [0,19)
N_PER_CORE = 19
N_PAIRS = N_PER_CORE + 2         # r-pair slab tiles per core (pairs i..i+2 per slot)

B_W, B_M1, B_M2, B_N = 180, 48, 15, 7
WIN_M2 = 60 + 2 * B_M2   # 90
WIN_M1 = 60 + 2 * B_M1   # 156 ; par-fused Kc=60 (per-chunk psum blocks)
WIN_W = 360              # tail-folded full circle (W split into k-pieces)
N_CORES = 8

F32 = np.float32
# Precision plan: polar (W/M1/M2) e4m3 x e4m3 DoubleRow as in v1. N region
# e4m3 pairs (value+residual) for both x and masters, all DR. x pre-scaled
# by XS, masters by MSCALE; output divided back.
IO_DTYPE = "float16"
MSCALE = 131072.0
XS = 2.0


def _cc_lats(n):
    return np.pi * np.arange(n) / (n - 1)


def _entries(psi_seg, psi_lat, psi_lon, psi_val, quad_weights):
    seg = np.asarray(psi_seg); lat = np.asarray(psi_lat)
    lon = np.asarray(psi_lon); val = np.asarray(psi_val, dtype=np.float64)
    qw = np.asarray(quad_weights, dtype=np.float64).reshape(-1)
    t = seg % T
    k = seg // T
    h = lat
    r = h - 2 * t + 2
    par = lon % 2
    e = lon // 2
    e_s = np.where(e < 180, e, e - 360)
    v = (val * qw[h]).astype(F32)
    return t, k, h, r, par, e_s, v


def _derive_tables(ent):
    """Per-slot-position band widths (identical across cores, data-derived)."""
    t, k, h, r, par, e_s, v = ent
    ext = np.zeros(T, dtype=int)
    np.maximum.at(ext, t, np.abs(e_s))
    n_bi = [max(int(ext[N_T0 + N_PER_CORE * c + i]) for c in range(N_CORES))
            for i in range(N_PER_CORE)]
    b_m2 = [max(int(ext[tv]) for tv in M2A_TS), max(int(ext[tv]) for tv in M2B_TS)]
    assert max(n_bi) <= B_N and max(b_m2) <= B_M2
    assert max(int(ext[tv]) for tv in M1_TS) <= B_M1
    return {"n_bi": n_bi, "b_m2": b_m2}


def _w_pieces(ci):
    """Two W k-pieces per core: (t, k) or None (dummy)."""
    if ci < 4:
        return [(W_TS[ci], 0), (W_TS[ci], 1)]
    return [(W_TS[ci - 4], 2), None]


def _core_assignment(ci):
    """slot -> t (or None for dummy). slots: 0=W,1=M1,2=M2a,3=M2b,4..22=N."""
    ts = [None] * 23
    if ci < len(W_TS):
        ts[0] = W_TS[ci]
    ts[1] = M1_TS[ci]
    ts[2] = M2A_TS[ci]
    ts[3] = M2B_TS[ci]
    for i in range(N_PER_CORE):
        ts[4 + i] = N_T0 + N_PER_CORE * ci + i
    return ts


def _precompute_globals(xbc, ent):
    """One-time global tensors: reversed parity-split x (fp32, cast at pack
    time) + class master banks (N fp32 fused; M1/W/M2 e4m3 DR-paired)."""
    import ml_dtypes
    E4 = ml_dtypes.float8_e4m3
    ms = MSCALE
    t, k, h, r, par, e_s, v = ent
    midx = (-np.arange(M360)) % M360
    # XR[par][h, m_rel(60), chunk(6)... stored [h, 6s, 60m, 128bc]
    XRE = np.ascontiguousarray(
        xbc[:, :, 2 * midx].transpose(1, 2, 0).reshape(H, 6, 60, BC))
    XRO = np.ascontiguousarray(
        xbc[:, :, 2 * midx + 1].transpose(1, 2, 0).reshape(H, 6, 60, BC))

    mr60 = np.arange(60)

    def fused_bank(t_list, Bc, width):
        tidx = {tv: i for i, tv in enumerate(t_list)}
        bank = np.zeros((len(t_list), 6, 120, 3 * width), dtype=F32)
        sel = np.isin(t, t_list)
        ti = np.array([tidx[tv] for tv in t[sel]])
        rr, pp, kk, ee, vv = r[sel], par[sel], k[sel], e_s[sel], v[sel]
        rows = (pp[:, None] * 60 + mr60[None, :]).ravel()
        jj = (mr60[None, :] + ee[:, None] + Bc).ravel()
        cols = jj * 3 + np.repeat(kk, 60)
        np.add.at(bank, (np.repeat(ti, 60), np.repeat(rr, 60), rows, cols),
                  np.repeat(vv, 60))
        # -> [n, 120p, 6r, 3W] contiguous, scaled
        return np.ascontiguousarray(bank.transpose(0, 2, 1, 3) * ms)

    # N masters stay fp32 here; per-core packing cuts per-slot width and
    # quantizes to e4m3 value+residual pairs.
    bank_N = fused_bank(list(range(N_T0, N_T0 + N_PER_CORE * N_CORES)),
                        B_N, 60 + 2 * B_N)
    bank_M2 = fused_bank(M2A_TS + M2B_TS, B_M2, WIN_M2).reshape(
        len(M2A_TS + M2B_TS), 120, 3, 2, 3 * WIN_M2).astype(E4)
    # M1: [n, 120, 6r, 468] -> DR pairs [n, 120, 3pair, 2plane, 468] e4m3
    bank_M1 = fused_bank(M1_TS, B_M1, WIN_M1)
    bank_M1 = bank_M1.reshape(len(M1_TS), 120, 3, 2, 3 * WIN_M1).astype(E4)

    # W pieces: single-k folded [t(4), k(3), 120p, 6r, 360]
    bank_W = np.zeros((4, 3, 6, 120, 360), dtype=F32)
    tidx = {tv: i for i, tv in enumerate(W_TS)}
    sel = np.isin(t, W_TS)
    ti = np.array([tidx[tv] for tv in t[sel]])
    rr, pp, kk, ee, vv = r[sel], par[sel], k[sel], e_s[sel], v[sel]
    rows = (pp[:, None] * 60 + mr60[None, :]).ravel()
    jj = ((mr60[None, :] + ee[:, None] + B_W) % 360).ravel()
    np.add.at(bank_W, (np.repeat(ti, 60), np.repeat(kk, 60), np.repeat(rr, 60),
                       rows, jj), np.repeat(vv, 60))
    # [t, k, 120p, 6r, 360] -> DR pairs [t, k, 120, 3, 2, 360] e4m3
    bank_W = np.ascontiguousarray(bank_W.transpose(0, 1, 3, 2, 4) * ms)
    bank_W = bank_W.reshape(4, 3, 120, 3, 2, 360).astype(E4)
    return XRE, XRO, bank_N, bank_M2, bank_M1, bank_W


def _pack_core_inputs(ci, glob, tables):
    import ml_dtypes
    E4 = ml_dtypes.float8_e4m3
    XRE, XRO, bank_N, bank_M2, bank_M1, bank_W = glob
    ts = _core_assignment(ci)

    # ---- polar slab: rows h = 2t+r-2 per region (W,M1,M2a,M2b), e4m3 pairs
    w_t = W_TS[ci % 4]
    polar_rows = []
    for slot, tv in ((0, w_t), (1, ts[1]), (2, ts[2]), (3, ts[3])):
        for rr in range(6):
            hh = 2 * tv + rr - 2
            polar_rows.append(hh if 0 <= hh < H else None)
    slabp = np.zeros((24, 128, 6, 128), dtype=F32)
    for j, hh in enumerate(polar_rows):
        if hh is None:
            continue
        slabp[j, 0:60] = XRE[hh].transpose(1, 0, 2)
        slabp[j, 60:120] = XRO[hh].transpose(1, 0, 2)
    slabp *= XS
    # -> e4m3 DR pair tiles, partition-major [128, 12 pair, 2, 6, 128]
    slab4 = np.ascontiguousarray(
        slabp.reshape(12, 2, 128, 6, 128).transpose(2, 0, 1, 3, 4)).astype(E4)

    # ---- N slab: 21 r-pair tiles [128parm, 2plane, 6chunk, 128bc], x4+xr
    tN0 = ts[4]
    slabn = np.zeros((N_PAIRS, 128, 2, 6, 128), dtype=F32)
    for pj in range(N_PAIRS):
        for pl in range(2):
            hh = 2 * (tN0 - 1 + pj) + pl
            # [6s,60m,128bc] -> parm=(par,60m): rows 0:60 E, 60:120 O
            slabn[pj, 0:60, pl] = XRE[hh].transpose(1, 0, 2)
            slabn[pj, 60:120, pl] = XRO[hh].transpose(1, 0, 2)
    slabn *= XS
    slab_x4 = slabn.astype(E4)
    slab_xr = (slabn - slab_x4.astype(F32)).astype(E4)
    # host layout [128, 21, 2, 6, 128] (partition-major)
    slab_x4 = np.ascontiguousarray(slab_x4.transpose(1, 0, 2, 3, 4))
    slab_xr = np.ascontiguousarray(slab_xr.transpose(1, 0, 2, 3, 4))

    # ---- N masters: per slot [120, 3rpair, 2plane, 3Wi] e4m3 pairs, concat
    n_bi = tables["n_bi"]
    m4_parts, mr_parts = [], []
    for i in range(N_PER_CORE):
        d = B_N - n_bi[i]
        Wi = 60 + 2 * n_bi[i]
        blk = bank_N[tN0 - N_T0 + i][:, :, 3 * d:3 * (d + Wi)]  # [120,6,3Wi]
        blk = blk.reshape(120, 3, 2, 3 * Wi)
        b4 = blk.astype(E4)
        br = (blk - b4.astype(F32)).astype(E4)
        m4_parts.append(np.ascontiguousarray(b4).reshape(120, -1))
        mr_parts.append(np.ascontiguousarray(br).reshape(120, -1))
    mN4 = np.ascontiguousarray(np.concatenate(m4_parts, axis=1))
    mNr = np.ascontiguousarray(np.concatenate(mr_parts, axis=1))

    # ---- polar masters (unchanged from v1)
    wp = _w_pieces(ci)
    mW = np.zeros((2, 120, 3, 2, 360), dtype=E4)
    widx = {tv: i for i, tv in enumerate(W_TS)}
    for j, piece in enumerate(wp):
        if piece is not None:
            tW, kW = piece
            mW[j] = bank_W[widx[tW], kW]
    mM1 = bank_M1[M1_TS.index(ts[1])]
    b_m2 = tables["b_m2"]
    m2idx = {tv: i for i, tv in enumerate(M2A_TS + M2B_TS)}
    m2_parts = []
    for sl in range(2):
        d = B_M2 - b_m2[sl]
        blk = bank_M2[m2idx[ts[2 + sl]]][:, :, :, 3 * d:3 * (d + 60 + 2 * b_m2[sl])]
        m2_parts.append(np.ascontiguousarray(blk).reshape(120, -1))
    mM2 = np.ascontiguousarray(np.concatenate(m2_parts, axis=1))
    return {"slab4": slab4, "slabx4": slab_x4, "slabxr": slab_xr,
            "mW": mW, "mM1": mM1, "mM2": mM2, "mN4": mN4, "mNr": mNr}


# ---------------- bass kernel emission ----------------

def _emit_kernel(tables):
    import concourse.tile as tile
    from concourse import bacc, mybir
    from contextlib import ExitStack

    DT = mybir.dt.float16
    E4 = mybir.dt.float8e4
    DR = mybir.MatmulPerfMode.DoubleRow
    F32d = mybir.dt.float32
    ADD = mybir.AluOpType.add

    n_bi = tables["n_bi"]
    b_m2 = tables["b_m2"]
    n_w = [60 + 2 * b for b in n_bi]
    m2_w = [60 + 2 * b for b in b_m2]
    n_off = np.concatenate([[0], np.cumsum([3 * 2 * 3 * w for w in n_w])]).tolist()
    m2_off = np.concatenate([[0], np.cumsum([6 * 3 * w for w in m2_w])]).tolist()

    nc = bacc.Bacc(None, target_bir_lowering=False)
    slab4_t = nc.dram_tensor("slab4", [128, 12, 2, 6, 128], E4,
                             kind="ExternalInput")
    slabx4_t = nc.dram_tensor("slabx4", [128, N_PAIRS, 2, 6, 128], E4,
                              kind="ExternalInput")
    slabxr_t = nc.dram_tensor("slabxr", [128, N_PAIRS, 2, 6, 128], E4,
                              kind="ExternalInput")
    mW_t = nc.dram_tensor("mW", [2, 120, 3, 2, 360], E4, kind="ExternalInput")
    mM1_t = nc.dram_tensor("mM1", [120, 3, 2, 3 * WIN_M1], E4,
                           kind="ExternalInput")
    mM2_t = nc.dram_tensor("mM2", [120, m2_off[-1]], E4, kind="ExternalInput")
    mN4_t = nc.dram_tensor("mN4", [120, n_off[-1]], E4, kind="ExternalInput")
    mNr_t = nc.dram_tensor("mNr", [120, n_off[-1]], E4, kind="ExternalInput")
    OUT_DT = DT
    out_t = nc.dram_tensor("out", [4, 128, 3, 360], OUT_DT, kind="ExternalOutput")
    outW_t = nc.dram_tensor("outW", [2, 128, 360], OUT_DT, kind="ExternalOutput")
    outN_t = nc.dram_tensor("outN", [N_PER_CORE, 128, 3, 512], OUT_DT,
                            kind="ExternalOutput")

    def retire_add(stage, ps, jj_len, p0, cov=None):
        """stage[:,:,p] (+)= ps cols; fresh p-cols (per cov map) use an ACT
        copy, already-covered cols a DVE add. cov=None -> all adds."""
        segs = []
        if cov is None:
            segs = [(0, jj_len, True)]
        else:
            a = 0
            while a < jj_len:
                p = (p0 + a) % 360
                old = bool(cov[p])
                b = a
                while b < jj_len and bool(cov[(p0 + b) % 360]) == old:
                    b += 1
                segs.append((a, b, old))
                a = b
            for a, b, old in segs:
                for q in range(a, b):
                    cov[(p0 + q) % 360] = True
        for a, b, old in segs:
            done = a
            while done < b:
                pstart = (p0 + done) % 360
                ln = min(b - done, 360 - pstart)
                s = ps[:, done * 3:(done + ln) * 3].rearrange("p (j k) -> p k j", k=3)
                dst = stage[:, :, pstart:pstart + ln]
                if old:
                    nc.vector.tensor_tensor(dst, s, dst, ADD)
                else:
                    nc.vector.tensor_copy(dst, s)
                done += ln

    with tile.TileContext(nc, pool_alloc_mode="queue") as tc:
        with ExitStack() as ctx:
            rows_pool = ctx.enter_context(tc.tile_pool(name="rows", bufs=1))
            stpool = ctx.enter_context(tc.tile_pool(name="stpool", bufs=7))
            nslab_pool = ctx.enter_context(tc.tile_pool(name="nslab", bufs=1))
            nmast_pool = ctx.enter_context(tc.tile_pool(name="nmast", bufs=1))

            # polar slab: one partition-major tile, chunked loads in
            # first-use order (M2 sl0 pair, rest of M2, W, M1)
            p4t = rows_pool.tile([128, 12, 2, 6, 128], E4, name="p4t", tag="xp")
            for a, b in ((6, 7), (7, 9), (9, 12), (0, 3), (3, 6)):
                nc.sync.dma_start(out=p4t[:, a:b], in_=slab4_t[:, a:b])
            pair_tiles = {pj: None for pj in range(12)}

            class _PairView:
                def __getitem__(self, pj):
                    return p4t[:, pj]
            pair_tiles = _PairView()

            # ---- N big tiles; loads split/staggered across the 3 queues.
            x4t = nslab_pool.tile([128, N_PAIRS, 2, 6, 128], E4, name="x4t",
                                  tag="nx4")
            xrt = nslab_pool.tile([128, N_PAIRS, 2, 6, 128], E4, name="xrt",
                                  tag="nxr")
            m4t = nmast_pool.tile([120, n_off[-1]], E4, name="m4t", tag="nm4")
            mrt = nmast_pool.tile([120, n_off[-1]], E4, name="mrt", tag="nmr")
            # Pool queue: N masters (then N out DMAs later)
            nc.gpsimd.dma_start(out=m4t[:, 0:n_off[6]], in_=mN4_t[:, 0:n_off[6]])
            nc.gpsimd.dma_start(out=mrt[:, 0:n_off[6]], in_=mNr_t[:, 0:n_off[6]])
            nc.gpsimd.dma_start(out=m4t[:, n_off[6]:], in_=mN4_t[:, n_off[6]:])
            nc.gpsimd.dma_start(out=mrt[:, n_off[6]:], in_=mNr_t[:, n_off[6]:])

            # ---- polar masters: all ACT DMA gens issued before ACT compute,
            # ordered by first-use time (M2 sl0 split in thirds for fast start)
            pmast_pool = ctx.enter_context(tc.tile_pool(name="pmast", bufs=1))
            m2ts = []
            for sl in range(2):
                Wsl = m2_w[sl]
                m2t = pmast_pool.tile([120, 6 * 3 * Wsl], E4, name=f"m2t{sl}",
                                      tag=f"m2_{sl}")
                if sl == 0:
                    for pr in range(3):
                        nc.scalar.dma_start(
                            out=m2t[:, pr * 6 * Wsl:(pr + 1) * 6 * Wsl],
                            in_=mM2_t[:, m2_off[sl] + pr * 6 * Wsl:
                                      m2_off[sl] + (pr + 1) * 6 * Wsl])
                else:
                    nc.scalar.dma_start(out=m2t[:],
                                        in_=mM2_t[:, m2_off[sl]:m2_off[sl + 1]])
                m2ts.append(m2t)
            mwts = []
            for j in range(2):
                mwt = pmast_pool.tile([120, 3, 2, 360], E4, name=f"mwt{j}",
                                      tag=f"mw_{j}")
                nc.scalar.dma_start(out=mwt[:], in_=mW_t[j])
                mwts.append(mwt)
            m1t = pmast_pool.tile([120, 3, 2, 3 * WIN_M1], E4, name="m1t",
                                  tag="m1")
            nc.scalar.dma_start(out=m1t[:], in_=mM1_t[:, :, :, :])
            nc.scalar.dma_start(out=x4t[:, 0:3], in_=slabx4_t[:, 0:3])
            nc.scalar.dma_start(out=xrt[:, 0:3], in_=slabxr_t[:, 0:3])

            pspool = ctx.enter_context(
                tc.tile_pool(name="pspool", bufs=4, space="PSUM"))
            npspool = ctx.enter_context(
                tc.tile_pool(name="npspool", bufs=4, space="PSUM"))
            # ---------------- M2 regions (slots 2,3): par-fused Kc=60 --------
            if True:
                for sl in range(2):
                    Wsl = m2_w[sl]
                    m2t = m2ts[sl]
                    stage2 = stpool.tile([128, 3, 360], OUT_DT, name=f"stage2_{sl}", tag="st")
                    cov2 = np.zeros(360, dtype=bool)
                    ps2 = [pspool.tile([128, 512], F32d, name=f"m2ps{sl}_{b}", tag="ps")
                           for b in range(3)]
                    for pr in range(3):
                        xt = pair_tiles[6 + sl * 3 + pr]
                        for c in range(6):
                            b, off = c // 2, (c % 2) * 60
                            rhs2 = m2t[:, pr * 6 * Wsl:(pr + 1) * 6 * Wsl]
                            rhs2 = rhs2.rearrange("p (two f) -> p two f", two=2)
                            nc.tensor.matmul(
                                ps2[b][:, off * 3:off * 3 + 3 * Wsl],
                                lhsT=xt[0:120, :, c, :],
                                rhs=rhs2,
                                start=(pr == 0 and c % 2 == 0),
                                stop=(pr == 2 and c % 2 == 1),
                                perf_mode=DR,
                            )
                    for b in range(3):
                        p0 = (120 * b - b_m2[sl] - 1) % 360
                        retire_add(stage2, ps2[b], 60 + Wsl, p0, cov2)
                    nc.scalar.dma_start(out=out_t[2 + sl], in_=stage2[:])

                # ---------------- W region: 2 single-k pieces, DR pairs ------
                if True:
                    wpairs = [pair_tiles[pj] for pj in range(3)]
                    for j in range(2):
                        mwt = mwts[j]
                        stageW = stpool.tile([128, 360], OUT_DT, name=f"stageW{j}", tag="stw")
                        for c in range(6):
                            wps = pspool.tile([128, 512], F32d, name=f"wps{j}_{c}", tag="ps")
                            for pr in range(3):
                                nc.tensor.matmul(
                                    wps[:, 0:360],
                                    lhsT=wpairs[pr][0:120, :, c, :],
                                    rhs=mwt[:, pr, :, :],
                                    start=(pr == 0),
                                    stop=(pr == 2),
                                    perf_mode=DR,
                                )
                            # p = (60c + jj' - 181) mod 360 over jj' in [0,360)
                            p0 = (60 * c - B_W - 1) % 360
                            done = 0
                            while done < 360:
                                pstart = (p0 + done) % 360
                                ln = min(360 - done, 360 - pstart)
                                dst = stageW[:, pstart:pstart + ln]
                                if c == 0:
                                    nc.vector.tensor_copy(dst, wps[:, done:done + ln])
                                else:
                                    nc.vector.tensor_tensor(dst, wps[:, done:done + ln],
                                                            dst, ADD)
                                done += ln
                        nc.scalar.dma_start(out=outW_t[j], in_=stageW[:])
                # ---------------- M1 region (slot 1): DR pairs, 6 psum blocks
                if True:
                    stage1 = stpool.tile([128, 3, 360], OUT_DT, name="stage1", tag="st")
                    cov1 = np.zeros(360, dtype=bool)
                    for rnd in range(2):
                        cs = (0, 1, 2) if rnd == 0 else (3, 4, 5)
                        ps1 = [pspool.tile([128, 512], F32d, name=f"m1ps{c}", tag="ps")
                               for c in cs]
                        for pr in range(3):
                            xt = pair_tiles[3 + pr]
                            for ci, c in enumerate(cs):
                                nc.tensor.matmul(
                                    ps1[ci][:, 0:3 * WIN_M1],
                                    lhsT=xt[0:120, :, c, :],
                                    rhs=m1t[:, pr, :, :],
                                    start=(pr == 0),
                                    stop=(pr == 2),
                                    perf_mode=DR,
                                )
                        for ci, c in enumerate(cs):
                            p0 = (60 * c - B_M1 - 1) % 360
                            retire_add(stage1, ps1[ci], WIN_M1, p0, cov1)
                    nc.scalar.dma_start(out=out_t[1], in_=stage1[:])

            # ---- N slab loads (queued behind polar pair loads on SP queue)
            nc.sync.dma_start(out=x4t[:, 3:8], in_=slabx4_t[:, 3:8])
            nc.sync.dma_start(out=xrt[:, 3:8], in_=slabxr_t[:, 3:8])
            nc.sync.dma_start(out=x4t[:, 8:14], in_=slabx4_t[:, 8:14])
            nc.sync.dma_start(out=xrt[:, 8:14], in_=slabxr_t[:, 8:14])
            nc.sync.dma_start(out=x4t[:, 14:N_PAIRS], in_=slabx4_t[:, 14:N_PAIRS])
            nc.sync.dma_start(out=xrt[:, 14:N_PAIRS], in_=slabxr_t[:, 14:N_PAIRS])

            # ---------------- N region: e4m3 pairs, DoubleRow, psum->DRAM ---
            if True:
                out_engs = [nc.gpsimd, nc.gpsimd]
                for i in range(N_PER_CORE):
                    Wi = n_w[i]
                    used = 180 + 3 * Wi
                    stN = stpool.tile([128, 3, used], OUT_DT, name=f"stN{i}",
                                      tag="stn")
                    for b in range(3):
                        ps = npspool.tile([128, 512], F32d, name=f"nps{i}_{b}",
                                          tag="nps")
                        for c in (2 * b, 2 * b + 1):
                            off = (c % 2) * 180
                            for gi, (xs, ms) in enumerate(
                                    ((x4t, m4t), (x4t, mrt), (xrt, m4t))):
                                for j in range(3):
                                    rhs = ms[:, n_off[i] + j * 2 * 3 * Wi:
                                             n_off[i] + (j + 1) * 2 * 3 * Wi]
                                    rhs = rhs.rearrange("p (two f) -> p two f", two=2)
                                    nc.tensor.matmul(
                                        ps[:, off:off + 3 * Wi],
                                        lhsT=xs[0:120, i + j, :, c, :],
                                        rhs=rhs,
                                        start=(c == 2 * b and gi == 0 and j == 0),
                                        stop=(c == 2 * b + 1 and gi == 2 and j == 2),
                                        perf_mode=DR,
                                    )
                        # retire bank b while bank b+1 matmuls run
                        if (i + b) % 2 == 0:
                            nc.scalar.copy(out=stN[:, b, :], in_=ps[:, 0:used])
                        else:
                            nc.vector.tensor_copy(stN[:, b, :], ps[:, 0:used])
                    if i >= N_PER_CORE - 2:
                        # tail: ship banks separately so the DMA of banks 0-1
                        # overlaps bank 2's matmuls/copy
                        for b in range(3):
                            nc.sync.dma_start(out=outN_t[i][:, b, 0:used],
                                              in_=stN[:, b, :])
                    else:
                        nc.gpsimd.dma_start(out=outN_t[i][:, :, 0:used],
                                            in_=stN[:])
    nc.finalize()
    return nc


_NC_CACHE = {}


def kernel(**inputs) -> np.ndarray:
    x = np.asarray(inputs["x"], dtype=F32)
    ent = _entries(inputs["psi_seg"], inputs["psi_lat"], inputs["psi_lon"],
                   inputs["psi_val"], inputs["quad_weights"])
    xbc = np.ascontiguousarray(x.reshape(BC, H, W))

    tables = _derive_tables(ent)
    glob = _precompute_globals(xbc, ent)
    in_maps = [_pack_core_inputs(ci, glob, tables) for ci in range(N_CORES)]

    key = (tuple(tables["n_bi"]), tuple(tables["b_m2"]))
    if _NC_CACHE.get("key") != key:
        _NC_CACHE["nc"] = _emit_kernel(tables)
        _NC_CACHE["key"] = key
    nc = _NC_CACHE["nc"]

    from concourse.bass_utils import run_bass_kernel_spmd
    try:
        res = run_bass_kernel_spmd(nc, in_maps, core_ids=list(range(N_CORES)))
    except (ImportError, ModuleNotFoundError):
        # BASS_TRACE set but the axon NTFF hook is unavailable in this env
        import os
        os.environ["BASS_NEVER_TRACE"] = "1"
        res = run_bass_kernel_spmd(nc, in_maps, core_ids=list(range(N_CORES)))
    global LAST_RESULTS
    LAST_RESULTS = res

    inv = np.float32(1.0 / (MSCALE * XS))
    n_bi = tables["n_bi"]
    full = np.zeros((BC, KS, T, P), dtype=F32)
    for ci in range(N_CORES):
        o = np.asarray(res.results[ci]["out"])
        ow = np.asarray(res.results[ci]["outW"])
        on = np.asarray(res.results[ci]["outN"])
        ts = _core_assignment(ci)
        for slot in (1, 2, 3):
            tv = ts[slot]
            if tv is not None:
                full[:, :, tv, :] = o[slot].astype(F32) * inv
        for j, piece in enumerate(_w_pieces(ci)):
            if piece is not None:
                tW, kW = piece
                full[:, kW, tW, :] = ow[j].astype(F32) * inv
        # N slots: host overlap-add of raw psum blocks
        for i in range(N_PER_CORE):
            tv = ts[4 + i]
            bi = n_bi[i]
            run = 120 + 2 * bi              # p-run length per block
            acc = np.zeros((BC, KS, P), dtype=F32)
            for b in range(3):
                blk = on[i, :, b, 0:3 * run].reshape(BC, run, KS)
                p0 = (120 * b - bi - 1) % 360
                pidx = (p0 + np.arange(run)) % 360
                np.add.at(acc, (slice(None), slice(None), pidx),
                          blk.transpose(0, 2, 1))
            full[:, :, tv, :] = acc * inv
    return full.reshape(B_, C_, KS, T, P)
